# revision 9
# baseline (speedup 1.0000x reference)
"""ColorConsistencyLoss on 8 Trainium2 NeuronCores.

Data-parallel over batch (2 images/core). Per core:
  host: pack rgb channel-planes into [128, 12800] bf16 (126 data rows =
        3 channels x 42 chunks of 12800, 2 pad rows of 1.0)
  device, per free-chunk of 2560 cols:
    mm1 (PE, bf16 x2 split):   t = (C1 + C2) . rgb      (RGB->XYZ)
    ACT Ln -> ACT Exp(1/3):    F = t^(1/3)              (cbrt, one table set)
    DVE:                       dF = F_pred - F_target   (-> bf16)
    mm2 (PE, bf16 x2 split):   v = (M1 + M2) . dF       (M = A^T A Lab mixing)
    DVE STT:                   acc[:, col] = sum(dF * v)
  host: loss = sum(acc) / N

C/M are split into bf16 value + bf16 residual so the systematic matrix
quantization error is ~(2^-9)^2; remaining errors are random per-element
bf16 rounding of the data (~3e-5 on the final scalar).
The f() linear branch (t <= T0, ~1e-5 of elements) is dropped: measured
effect ~2e-6 relative. The L-channel `where` is algebraically redundant.
"""
import os
import numpy as np

_B, _CH, _H, _W = 16, 3, 512, 512
_NCORES = 8
_IPC = _B // _NCORES            # images per core
_PIX = _IPC * _H * _W           # 524288 pixels per core per tensor
_NCHUNK = 42                    # chunks per channel -> 126 data rows
_CHUNK = 12800                  # padded chunk length = 25*512
_P = 128
_MMF = 512                      # matmul moving free dim
# free-dim chunks: 8x1536 + 512 tail (PSUM: 2x3-bank tps + 2x1-bank vps)
_FCHUNKS = [(i * 1536, 1536) for i in range(8)] + [(8 * 1536, 512)]
_NACC = 9                       # accumulator columns (one per free-chunk)

_XN, _ZN = 0.950456, 1.088754
_COEF = (
    (0.412453 / _XN, 0.357580 / _XN, 0.180423 / _XN),   # x from r,g,b
    (0.212671, 0.715160, 0.072169),                     # y
    (0.019334 / _ZN, 0.119193 / _ZN, 0.950227 / _ZN),   # z
)


def _bf16():
    import ml_dtypes
    return ml_dtypes.bfloat16


def _build_mats():
    """C (rgb->xyz) and M (=A^T A Lab mixing), each split into bf16 + bf16 residual."""
    bf16 = _bf16()
    C = np.zeros((_P, _P), np.float64)
    for oc in range(3):
        for ic in range(3):
            w = _COEF[oc][ic]
            for j in range(_NCHUNK):
                C[ic * _NCHUNK + j, oc * _NCHUNK + j] = w
    C[126, 126] = C[127, 127] = 1.0   # pad rows pass through (value 1.0)

    p, q, L = 500.0 / 255.0, 200.0 / 255.0, 1.16
    M = np.zeros((_P, _P), np.float64)
    for j in range(_NCHUNK):
        fx, fy, fz = j, _NCHUNK + j, 2 * _NCHUNK + j
        M[fx, fx] += p * p
        M[fx, fy] -= p * p
        M[fy, fx] -= p * p
        M[fy, fy] += L * L + p * p + q * q
        M[fy, fz] -= q * q
        M[fz, fy] -= q * q
        M[fz, fz] += q * q

    def split(A):
        A1 = A.astype(np.float32).astype(bf16)
        A2 = (A - A1.astype(np.float64)).astype(np.float32).astype(bf16)
        return A1, A2

    return split(C) + split(M)   # C1, C2, M1, M2


def _pack_core(arr):
    """[2,3,512,512] f32 -> [128, 12800] bf16 channel-chunk layout."""
    bf16 = _bf16()
    x = np.transpose(np.asarray(arr, np.float32), (1, 0, 2, 3)).reshape(_CH, _PIX)
    flat = np.ones((_CH, _NCHUNK * _CHUNK), np.float32)
    flat[:, :_PIX] = x
    out = np.ones((_P, _CHUNK), np.float32)
    out[:126] = flat.reshape(_CH * _NCHUNK, _CHUNK)
    return out.astype(bf16)


def _setup_act_tables():
    """Build a custom ACT table dir: one set (natural_log_exp_and_others) whose
    `ln` slot is re-bucketed to compute the exact piecewise CIE f(t)
    (cbrt above T0, tangent line below). One table load, one ACT pass."""
    import json
    import shutil
    import act_table_tool as att

    meta, bkt, ctl = att.load_set()
    meta2, bkt2 = att.patch_ln_to_f(meta, bkt, ctl)

    from neuronxcc.driver.Job import Job
    from neuronxcc.driver.jobs.support.FindActInfo import findActInfoFile
    src = findActInfoFile(Job.getPackageDir(), "gen3")
    srcdir = os.path.dirname(src)
    info = json.load(open(src))
    keep = [s for s in info["act_func_sets"]
            if s["name"] == "natural_log_exp_and_others"]
    assert keep, "natural_log_exp_and_others set not found"
    info["act_func_sets"] = keep

    d = "/tmp/act_custom"
    os.makedirs(d, exist_ok=True)
    s = keep[0]
    bkt2.astype(np.uint32).tofile(os.path.join(d, s["bkt_bin"]))
    shutil.copy(os.path.join(srcdir, s["ctrl_bin"]), os.path.join(d, s["ctrl_bin"]))
    with open(os.path.join(d, s["profile_json"]), "w") as f:
        json.dump(meta2, f)
    path = os.path.join(d, "act_info.json")
    with open(path, "w") as f:
        json.dump(info, f)
    os.environ["BASS_ACT_ROOT_JSON_PATH"] = path

    import concourse.bacc as bacc_mod
    import concourse.mybir as mybir
    tables = {
        s["name"]: {mybir.ActivationFunctionType.from_pwp(v)
                    for v in s["act"].keys()}
        for s in keep
    }
    bacc_mod.get_activation_tables = lambda arch: dict(tables)


_PROGRAM = None


def _patch_ldw_opt():
    import concourse.bass_utils as bu
    if getattr(bu, "_ldw_opt_patched", False):
        return
    orig = bu.run_command

    def run_command(argv, **kwargs):
        return orig(argv, **kwargs)  # ldw-opt rejected by walrus; keep as-is
        return orig(argv, **kwargs)

    bu.run_command = run_command
    bu._ldw_opt_patched = True


def _build_program():
    import concourse.bacc as bacc
    import concourse.tile as tile
    from concourse import mybir

    _setup_act_tables()
    _patch_ldw_opt()

    f32, bf = mybir.dt.float32, mybir.dt.bfloat16
    AF = mybir.ActivationFunctionType
    ALU = mybir.AluOpType

    nc = bacc.Bacc("TRN2", target_bir_lowering=False, debug=False)
    c1 = nc.dram_tensor("c1", [_P, _P], bf, kind="ExternalInput")
    c2 = nc.dram_tensor("c2", [_P, _P], bf, kind="ExternalInput")
    m1 = nc.dram_tensor("m1", [_P, _P], bf, kind="ExternalInput")
    m2 = nc.dram_tensor("m2", [_P, _P], bf, kind="ExternalInput")
    xp = nc.dram_tensor("xp", [_P, _CHUNK], bf, kind="ExternalInput")
    xt = nc.dram_tensor("xt", [_P, _CHUNK], bf, kind="ExternalInput")
    acc_out = nc.dram_tensor("acc_out", [_P, _NACC], f32, kind="ExternalOutput")

    with tile.TileContext(nc) as tc:
        with tc.tile_pool(name="consts", bufs=1) as consts, \
             tc.tile_pool(name="rgbp", bufs=4) as rgbp, \
             tc.tile_pool(name="sp", bufs=2) as sp, \
             tc.tile_pool(name="fp", bufs=2) as fpool, \
             tc.tile_pool(name="ft", bufs=2) as ftpool, \
             tc.tile_pool(name="dfp", bufs=2) as dfp, \
             tc.tile_pool(name="scrp", bufs=2) as scrp, \
             tc.tile_pool(name="accp", bufs=1) as accp, \
             tc.tile_pool(name="tpsp", bufs=2, space="PSUM") as tpsp:
            c1t = consts.tile([_P, _P], bf)
            c2t = consts.tile([_P, _P], bf)
            m1t = consts.tile([_P, _P], bf)
            m2t = consts.tile([_P, _P], bf)
            nc.sync.dma_start(out=c1t, in_=c1[:, :])
            nc.sync.dma_start(out=c2t, in_=c2[:, :])
            nc.sync.dma_start(out=m1t, in_=m1[:, :])
            nc.sync.dma_start(out=m2t, in_=m2[:, :])
            acc = accp.tile([_P, _NACC], f32)

            for fc, (lo, fw) in enumerate(_FCHUNKS):
                nmm = fw // _MMF
                Fs = {}
                for which, src in (("p", xp), ("t", xt)):
                    rgb = rgbp.tile([_P, fw], bf, tag="rgb")
                    nc.sync.dma_start(out=rgb, in_=src[:, lo:lo + fw])
                    tps = tpsp.tile([_P, fw], f32, tag="tps")
                    for wt in (c1t, c2t):
                        for j in range(nmm):
                            sl = slice(j * _MMF, (j + 1) * _MMF)
                            nc.tensor.matmul(tps[:, sl], wt[:, :], rgb[:, sl],
                                             start=wt is c1t, stop=wt is c2t)
                    pool = fpool if which == "p" else ftpool
                    F = pool.tile([_P, fw], f32, tag="F" + which)
                    # Ln slot is re-bucketed to the piecewise CIE f(t)
                    nc.scalar.activation(F, tps, AF.Ln)
                    Fs[which] = F

                dF = dfp.tile([_P, fw], bf, tag="dF")
                nc.vector.tensor_tensor(out=dF, in0=Fs["p"], in1=Fs["t"],
                                        op=ALU.subtract)
                vps = tpsp.tile([_P, fw], f32, tag="tps")
                for wt in (m1t, m2t):
                    for j in range(nmm):
                        sl = slice(j * _MMF, (j + 1) * _MMF)
                        nc.tensor.matmul(vps[:, sl], wt[:, :], dF[:, sl],
                                         start=wt is m1t, stop=wt is m2t)
                scratch = scrp.tile([_P, fw], f32, tag="scr")
                nc.vector.scalar_tensor_tensor(
                    out=scratch,
                    in0=dF[:, :],
                    scalar=1.0,
                    in1=vps[:, :],
                    op0=ALU.mult,
                    op1=ALU.mult,
                    accum_out=acc[:, fc:fc + 1],
                )

            nc.sync.dma_start(out=acc_out[:, :], in_=acc)

    nc.compile()
    return nc


def _get_program():
    global _PROGRAM
    if _PROGRAM is None:
        _PROGRAM = _build_program()
    return _PROGRAM


def _make_in_maps(pred, target):
    C1, C2, M1, M2 = _build_mats()
    pred = np.asarray(pred, np.float32)
    target = np.asarray(target, np.float32)
    in_maps = []
    for c in range(_NCORES):
        sl = slice(c * _IPC, (c + 1) * _IPC)
        in_maps.append({
            "c1": C1, "c2": C2, "m1": M1, "m2": M2,
            "xp": _pack_core(pred[sl]),
            "xt": _pack_core(target[sl]),
        })
    return in_maps


def kernel(pred, target):
    from concourse.bass_utils import run_bass_kernel_spmd

    nc = _get_program()
    in_maps = _make_in_maps(pred, target)
    res = run_bass_kernel_spmd(nc, in_maps, core_ids=list(range(_NCORES)))
    total = sum(r["acc_out"].astype(np.float64).sum() for r in res.results)
    loss = total / float(_B * _CH * _H * _W)
    return np.float32(loss)


if __name__ == "__main__":
    rng = np.random.default_rng(0)
    pred = rng.uniform(0, 1, (_B, _CH, _H, _W)).astype(np.float32)
    target = rng.uniform(0, 1, (_B, _CH, _H, _W)).astype(np.float32)
    print("loss:", kernel(pred, target))


# revision 10
# speedup vs baseline: 1.6054x; 1.6054x over previous
"""ColorConsistencyLoss on 8 Trainium2 NeuronCores.

Data-parallel over batch (2 images/core). Per core:
  host: pack rgb channel-planes into [128, 12800] bf16 (126 data rows =
        3 channels x 42 chunks of 12800, 2 pad rows of 1.0)
  device, per free-chunk of 2560 cols:
    mm1 (PE, bf16 x2 split):   t = (C1 + C2) . rgb      (RGB->XYZ)
    ACT Ln -> ACT Exp(1/3):    F = t^(1/3)              (cbrt, one table set)
    DVE:                       dF = F_pred - F_target   (-> bf16)
    mm2 (PE, bf16 x2 split):   v = (M1 + M2) . dF       (M = A^T A Lab mixing)
    DVE STT:                   acc[:, col] = sum(dF * v)
  host: loss = sum(acc) / N

C/M are split into bf16 value + bf16 residual so the systematic matrix
quantization error is ~(2^-9)^2; remaining errors are random per-element
bf16 rounding of the data (~3e-5 on the final scalar).
The f() linear branch (t <= T0, ~1e-5 of elements) is dropped: measured
effect ~2e-6 relative. The L-channel `where` is algebraically redundant.
"""
import os
import numpy as np

_B, _CH, _H, _W = 16, 3, 512, 512
_NCORES = 8
_IPC = _B // _NCORES            # images per core
_PIX = _IPC * _H * _W           # 524288 pixels per core per tensor
_NCHUNK = 42                    # chunks per channel -> 126 data rows
_CHUNK = 12800                  # padded chunk length = 25*512
_P = 128
_MMF = 512                      # matmul moving free dim
# free-dim chunks: 8x1536 + 512 tail (PSUM: 2x3-bank tps + 2x1-bank vps)
_FCHUNKS = [(i * 1536, 1536) for i in range(8)] + [(8 * 1536, 512)]
_NACC = _CHUNK // _MMF          # 25 accumulator columns (one per 512-slice)

_XN, _ZN = 0.950456, 1.088754
_COEF = (
    (0.412453 / _XN, 0.357580 / _XN, 0.180423 / _XN),   # x from r,g,b
    (0.212671, 0.715160, 0.072169),                     # y
    (0.019334 / _ZN, 0.119193 / _ZN, 0.950227 / _ZN),   # z
)


def _bf16():
    import ml_dtypes
    return ml_dtypes.bfloat16


def _build_mats():
    """C (rgb->xyz) and M (=A^T A Lab mixing), each split into bf16 + bf16 residual."""
    bf16 = _bf16()
    C = np.zeros((_P, _P), np.float64)
    for oc in range(3):
        for ic in range(3):
            w = _COEF[oc][ic]
            for j in range(_NCHUNK):
                C[ic * _NCHUNK + j, oc * _NCHUNK + j] = w
    C[126, 126] = C[127, 127] = 1.0   # pad rows pass through (value 1.0)

    p, q, L = 500.0 / 255.0, 200.0 / 255.0, 1.16
    M = np.zeros((_P, _P), np.float64)
    for j in range(_NCHUNK):
        fx, fy, fz = j, _NCHUNK + j, 2 * _NCHUNK + j
        M[fx, fx] += p * p
        M[fx, fy] -= p * p
        M[fy, fx] -= p * p
        M[fy, fy] += L * L + p * p + q * q
        M[fy, fz] -= q * q
        M[fz, fy] -= q * q
        M[fz, fz] += q * q

    def split(A):
        A1 = A.astype(np.float32).astype(bf16)
        A2 = (A - A1.astype(np.float64)).astype(np.float32).astype(bf16)
        return A1, A2

    return split(C) + split(M)   # C1, C2, M1, M2


def _pack_core(arr):
    """[2,3,512,512] f32 -> [128, 12800] bf16 channel-chunk layout."""
    bf16 = _bf16()
    x = np.transpose(np.asarray(arr, np.float32), (1, 0, 2, 3)).reshape(_CH, _PIX)
    flat = np.ones((_CH, _NCHUNK * _CHUNK), np.float32)
    flat[:, :_PIX] = x
    out = np.ones((_P, _CHUNK), np.float32)
    out[:126] = flat.reshape(_CH * _NCHUNK, _CHUNK)
    return out.astype(bf16)


def _setup_act_tables():
    """Build a custom ACT table dir: one set (natural_log_exp_and_others) whose
    `ln` slot is re-bucketed to compute the exact piecewise CIE f(t)
    (cbrt above T0, tangent line below). One table load, one ACT pass."""
    import json
    import shutil
    import act_table_tool as att

    meta, bkt, ctl = att.load_set()
    meta2, bkt2 = att.patch_ln_to_f(meta, bkt, ctl)

    from neuronxcc.driver.Job import Job
    from neuronxcc.driver.jobs.support.FindActInfo import findActInfoFile
    src = findActInfoFile(Job.getPackageDir(), "gen3")
    srcdir = os.path.dirname(src)
    info = json.load(open(src))
    keep = [s for s in info["act_func_sets"]
            if s["name"] == "natural_log_exp_and_others"]
    assert keep, "natural_log_exp_and_others set not found"
    info["act_func_sets"] = keep

    d = "/tmp/act_custom"
    os.makedirs(d, exist_ok=True)
    s = keep[0]
    bkt2.astype(np.uint32).tofile(os.path.join(d, s["bkt_bin"]))
    shutil.copy(os.path.join(srcdir, s["ctrl_bin"]), os.path.join(d, s["ctrl_bin"]))
    with open(os.path.join(d, s["profile_json"]), "w") as f:
        json.dump(meta2, f)
    path = os.path.join(d, "act_info.json")
    with open(path, "w") as f:
        json.dump(info, f)
    os.environ["BASS_ACT_ROOT_JSON_PATH"] = path

    import concourse.bacc as bacc_mod
    import concourse.mybir as mybir
    tables = {
        s["name"]: {mybir.ActivationFunctionType.from_pwp(v)
                    for v in s["act"].keys()}
        for s in keep
    }
    bacc_mod.get_activation_tables = lambda arch: dict(tables)


_PROGRAM = None


def _patch_ldw_opt():
    import concourse.bass_utils as bu
    if getattr(bu, "_ldw_opt_patched", False):
        return
    orig = bu.run_command

    def run_command(argv, **kwargs):
        return orig(argv, **kwargs)  # ldw-opt rejected by walrus; keep as-is
        return orig(argv, **kwargs)

    bu.run_command = run_command
    bu._ldw_opt_patched = True


def _build_program():
    import concourse.bacc as bacc
    import concourse.tile as tile
    from concourse import mybir

    _setup_act_tables()
    _patch_ldw_opt()

    f32, bf = mybir.dt.float32, mybir.dt.bfloat16
    AF = mybir.ActivationFunctionType
    ALU = mybir.AluOpType

    nc = bacc.Bacc("TRN2", target_bir_lowering=False, debug=False)
    c1 = nc.dram_tensor("c1", [_P, _P], bf, kind="ExternalInput")
    c2 = nc.dram_tensor("c2", [_P, _P], bf, kind="ExternalInput")
    m1 = nc.dram_tensor("m1", [_P, _P], bf, kind="ExternalInput")
    m2 = nc.dram_tensor("m2", [_P, _P], bf, kind="ExternalInput")
    xp = nc.dram_tensor("xp", [_P, _CHUNK], bf, kind="ExternalInput")
    xt = nc.dram_tensor("xt", [_P, _CHUNK], bf, kind="ExternalInput")
    acc_out = nc.dram_tensor("acc_out", [_P, _NACC], f32, kind="ExternalOutput")

    with tile.TileContext(nc) as tc:
        with tc.tile_pool(name="consts", bufs=1) as consts, \
             tc.tile_pool(name="rgbp", bufs=4) as rgbp, \
             tc.tile_pool(name="sp", bufs=2) as sp, \
             tc.tile_pool(name="fp", bufs=2) as fpool, \
             tc.tile_pool(name="ft", bufs=2) as ftpool, \
             tc.tile_pool(name="dfp", bufs=2) as dfp, \
             tc.tile_pool(name="scrp", bufs=2) as scrp, \
             tc.tile_pool(name="accp", bufs=1) as accp, \
             tc.tile_pool(name="tpsp", bufs=2, space="PSUM") as tpsp, \
             tc.tile_pool(name="vpsp", bufs=2, space="PSUM") as vpsp:
            c1t = consts.tile([_P, _P], bf)
            c2t = consts.tile([_P, _P], bf)
            m1t = consts.tile([_P, _P], bf)
            m2t = consts.tile([_P, _P], bf)
            nc.sync.dma_start(out=c1t, in_=c1[:, :])
            nc.sync.dma_start(out=c2t, in_=c2[:, :])
            nc.sync.dma_start(out=m1t, in_=m1[:, :])
            nc.sync.dma_start(out=m2t, in_=m2[:, :])
            acc = accp.tile([_P, _NACC], f32)

            for fc, (lo, fw) in enumerate(_FCHUNKS):
                nmm = fw // _MMF
                Fs = {}
                for which, src in (("p", xp), ("t", xt)):
                    rgb = rgbp.tile([_P, fw], bf, tag="rgb")
                    nc.sync.dma_start(out=rgb, in_=src[:, lo:lo + fw])
                    tps = tpsp.tile([_P, fw], f32, tag="tps")
                    for j in range(nmm):
                        sl = slice(j * _MMF, (j + 1) * _MMF)
                        nc.tensor.matmul(tps[:, sl], c1t[:, :], rgb[:, sl],
                                         start=True, stop=False)
                        nc.tensor.matmul(tps[:, sl], c2t[:, :], rgb[:, sl],
                                         start=False, stop=True)
                    pool = fpool if which == "p" else ftpool
                    F = pool.tile([_P, fw], f32, tag="F" + which)
                    # Ln slot is re-bucketed to the piecewise CIE f(t)
                    nc.scalar.activation(F, tps, AF.Ln)
                    Fs[which] = F

                dF = dfp.tile([_P, fw], bf, tag="dF")
                nc.vector.tensor_tensor(out=dF, in0=Fs["p"], in1=Fs["t"],
                                        op=ALU.subtract)
                for j in range(nmm):
                    sl = slice(j * _MMF, (j + 1) * _MMF)
                    vps = vpsp.tile([_P, _MMF], f32, tag="vps")
                    nc.tensor.matmul(vps[:, :], m1t[:, :], dF[:, sl],
                                     start=True, stop=False)
                    nc.tensor.matmul(vps[:, :], m2t[:, :], dF[:, sl],
                                     start=False, stop=True)
                    scratch = scrp.tile([_P, _MMF], f32, tag="scr")
                    nc.vector.scalar_tensor_tensor(
                        out=scratch,
                        in0=dF[:, sl],
                        scalar=1.0,
                        in1=vps[:, :],
                        op0=ALU.mult,
                        op1=ALU.mult,
                        accum_out=acc[:, (lo // _MMF) + j:(lo // _MMF) + j + 1],
                    )

            nc.sync.dma_start(out=acc_out[:, :], in_=acc)

    nc.compile()
    return nc


def _get_program():
    global _PROGRAM
    if _PROGRAM is None:
        _PROGRAM = _build_program()
    return _PROGRAM


def _make_in_maps(pred, target):
    C1, C2, M1, M2 = _build_mats()
    pred = np.asarray(pred, np.float32)
    target = np.asarray(target, np.float32)
    in_maps = []
    for c in range(_NCORES):
        sl = slice(c * _IPC, (c + 1) * _IPC)
        in_maps.append({
            "c1": C1, "c2": C2, "m1": M1, "m2": M2,
            "xp": _pack_core(pred[sl]),
            "xt": _pack_core(target[sl]),
        })
    return in_maps


def kernel(pred, target):
    from concourse.bass_utils import run_bass_kernel_spmd

    nc = _get_program()
    in_maps = _make_in_maps(pred, target)
    res = run_bass_kernel_spmd(nc, in_maps, core_ids=list(range(_NCORES)))
    total = sum(r["acc_out"].astype(np.float64).sum() for r in res.results)
    loss = total / float(_B * _CH * _H * _W)
    return np.float32(loss)


if __name__ == "__main__":
    rng = np.random.default_rng(0)
    pred = rng.uniform(0, 1, (_B, _CH, _H, _W)).astype(np.float32)
    target = rng.uniform(0, 1, (_B, _CH, _H, _W)).astype(np.float32)
    print("loss:", kernel(pred, target))
